# revision 9
# baseline (speedup 1.0000x reference)
"""3x3 median filter (reflect padding) on Trainium2, 8-core data parallel.

Layout (per core, 4 images):
  partition p = b*32 + g
    b in 0..3  : image index within the core's batch shard
    g in 0..31 : group of 7 consecutive output rows
  Work is split into 2 row-chunks per group (3 + 4 output rows).  Each
  partition's slab holds (R+2) full-width rows of (224+2)px x 3ch fp32,
  so vertical (stride F) and horizontal (stride 3) neighbor access are
  free-dim offsets and each DRAM row moves as one contiguous 2688B run.

Median of 9 = med3( max3(col_lows), med3(col_meds), min3(col_highs) )
with each vertical column triple sorted once and shared across the three
horizontally adjacent windows.

Loads/stores are spread over three DMA streams (SP + ACT hardware DGE
queues and the GPSIMD software DGE).
"""

import sys

if "/opt/trn_rl_repo" not in sys.path:
    sys.path.insert(0, "/opt/trn_rl_repo")

import numpy as np

import concourse.bass as bass  # noqa: F401
import concourse.tile as tile
from concourse import bacc, mybir
from concourse.ap import AP
from concourse.bass_utils import run_bass_kernel_spmd

F32 = mybir.dt.float32
MIN = mybir.AluOpType.min
MAX = mybir.AluOpType.max

B, H, W, C = 32, 224, 224, 3
NCORES = 8
BPC = B // NCORES      # 4 images per core
NG, GR = 32, 7         # row-groups per image, rows per group
CHUNK_ROWS = (3, 4)    # output rows per chunk within each group
F = (W + 2) * C        # 678 floats per slab row incl. horizontal halo
SC = W * C             # 672 output floats per row
WC = W * C             # 672
IMG = H * WC

_CACHE = {}


def _build_kernel(tc, y, x):
    nc = tc.nc

    with tc.tile_pool(name="sb", bufs=1) as sb:
        r0g = 0  # first output row of this chunk within its group
        for chunk, R in enumerate(CHUNK_ROWS):
            SRR = R + 2
            S = sb.tile([128, SRR, F], F32, tag="s", bufs=2, name=f"S{chunk}")

            # ---- loads -------------------------------------------------
            # slab rows = image rows 7g + r0g - 1 .. 7g + r0g + R
            first = chunk == 0
            last = chunk == len(CHUNK_ROWS) - 1
            for b in range(BPC):
                base = b * IMG
                p0 = b * 32
                # bulk: all groups with full vertical halo in-bounds
                g0 = 1 if first else 0
                g1 = NG - 1 if last else NG
                off = base + ((g0 * GR) + r0g - 1) * WC
                dims = [[GR * WC, g1 - g0], [WC, SRR], [1, WC]]
                dp0, dp1 = p0 + g0, p0 + g1
                if b == 3:
                    # split image 3's bulk between the two HW queues
                    half = (g1 - g0) // 2
                    s1 = AP(x.tensor, off, [[GR * WC, half], [WC, SRR], [1, WC]])
                    nc.sync.dma_start(S[dp0:dp0 + half, :, 3:675], s1)
                    s2 = AP(x.tensor, off + half * GR * WC,
                            [[GR * WC, g1 - g0 - half], [WC, SRR], [1, WC]])
                    nc.scalar.dma_start(S[dp0 + half:dp1, :, 3:675], s2)
                else:
                    eng = (nc.sync, nc.scalar, nc.gpsimd)[b]
                    eng.dma_start(S[dp0:dp1, :, 3:675],
                                  AP(x.tensor, off, dims))
                eeng = nc.sync if b < 2 else nc.scalar
                if first:
                    # group 0 lacks image row -1: load rows 0..R, reflect 1
                    e = AP(x.tensor, base, [[WC, SRR - 1], [1, WC]])
                    eeng.dma_start(S[p0:p0 + 1, 1:SRR, 3:675], e)
                    r = AP(x.tensor, base + WC, [[1, WC]])
                    eeng.dma_start(S[p0:p0 + 1, 0:1, 3:675], r)
                if last:
                    # group 31 lacks image row 224: rows 219..223, reflect 222
                    row = (NG - 1) * GR + r0g - 1
                    e = AP(x.tensor, base + row * WC, [[WC, SRR - 1], [1, WC]])
                    eeng.dma_start(S[p0 + 31:p0 + 32, 0:SRR - 1, 3:675], e)
                    r = AP(x.tensor, base + (H - 2) * WC, [[1, WC]])
                    eeng.dma_start(S[p0 + 31:p0 + 32, SRR - 1:SRR, 3:675], r)

            # horizontal reflect: col -1 <- col 1, col 224 <- col 222
            nc.vector.tensor_copy(S[:, :, 0:3], S[:, :, 6:9])
            nc.vector.tensor_copy(S[:, :, 675:678], S[:, :, 669:672])

            # ---- stage 1: vertical column sort -------------------------
            P = sb.tile([128, R, F], F32, tag="p", name=f"P{chunk}")
            Q = sb.tile([128, R, F], F32, tag="q", name=f"Q{chunk}")
            nc.vector.tensor_tensor(P[:], S[:, 0:R, :], S[:, 1:R + 1, :], MIN)
            nc.vector.tensor_tensor(Q[:], S[:, 0:R, :], S[:, 1:R + 1, :], MAX)

            LO = sb.tile([128, R, F], F32, tag="lo", name=f"LO{chunk}")
            T = sb.tile([128, R, F], F32, tag="t", name=f"T{chunk}")
            nc.vector.tensor_tensor(LO[:], P[:], S[:, 2:SRR, :], MIN)
            nc.vector.tensor_tensor(T[:], Q[:], S[:, 2:SRR, :], MIN)
            # MED (in T): max(P, min(Q, S+2))
            nc.vector.tensor_tensor(T[:], P[:], T[:], MAX)
            # HI (in Q): max(Q, S+2)
            nc.vector.tensor_tensor(Q[:], Q[:], S[:, 2:SRR, :], MAX)
            HI = Q

            # ---- stage 2: horizontal merge -----------------------------
            U = sb.tile([128, R, F - 3], F32, tag="u", name=f"U{chunk}")
            nc.vector.tensor_tensor(U[:], LO[:, :, 0:F - 3], LO[:, :, 3:F], MAX)
            nc.vector.tensor_tensor(U[:, :, 0:SC], U[:, :, 0:SC],
                                    LO[:, :, 6:F], MAX)
            A = U  # max3 of lows

            V = sb.tile([128, R, F - 3], F32, tag="v", name=f"V{chunk}")
            nc.vector.tensor_tensor(V[:], HI[:, :, 0:F - 3], HI[:, :, 3:F], MIN)
            nc.vector.tensor_tensor(V[:, :, 0:SC], V[:, :, 0:SC],
                                    HI[:, :, 6:F], MIN)
            Cc = V  # min3 of highs

            Sm = sb.tile([128, R, F - 3], F32, tag="sm", name=f"Sm{chunk}")
            Tm = sb.tile([128, R, F - 3], F32, tag="tm", name=f"Tm{chunk}")
            nc.vector.tensor_tensor(Sm[:], T[:, :, 0:F - 3], T[:, :, 3:F], MIN)
            nc.vector.tensor_tensor(Tm[:], T[:, :, 0:F - 3], T[:, :, 3:F], MAX)
            nc.vector.tensor_tensor(Tm[:, :, 0:SC], Tm[:, :, 0:SC],
                                    T[:, :, 6:F], MIN)
            nc.vector.tensor_tensor(Sm[:, :, 0:SC], Sm[:, :, 0:SC],
                                    Tm[:, :, 0:SC], MAX)
            Bm = Sm  # med3 of meds

            # ---- final med3(A, B, C) -----------------------------------
            M1 = sb.tile([128, R, SC], F32, tag="m1", bufs=2, name=f"M1{chunk}")
            nc.vector.tensor_tensor(M1[:], A[:, :, 0:SC], Bm[:, :, 0:SC], MIN)
            nc.vector.tensor_tensor(A[:, :, 0:SC], A[:, :, 0:SC],
                                    Bm[:, :, 0:SC], MAX)
            nc.vector.tensor_tensor(Cc[:, :, 0:SC], A[:, :, 0:SC],
                                    Cc[:, :, 0:SC], MIN)
            nc.vector.tensor_tensor(M1[:], M1[:], Cc[:, :, 0:SC], MAX)

            # ---- store -------------------------------------------------
            for b in range(BPC):
                p0 = b * 32
                off = b * IMG + r0g * WC
                if b == 3:
                    d1 = AP(y.tensor, off, [[GR * WC, 16], [WC, R], [1, SC]])
                    nc.scalar.dma_start(d1, M1[p0:p0 + 16, :, :])
                    d2 = AP(y.tensor, off + 16 * GR * WC,
                            [[GR * WC, 16], [WC, R], [1, SC]])
                    nc.sync.dma_start(d2, M1[p0 + 16:p0 + 32, :, :])
                else:
                    eng = (nc.scalar, nc.sync, nc.gpsimd)[b]
                    dst = AP(y.tensor, off, [[GR * WC, NG], [WC, R], [1, SC]])
                    eng.dma_start(dst, M1[p0:p0 + 32, :, :])

            r0g += R


def _build():
    if "nc" in _CACHE:
        return _CACHE["nc"]
    nc = bacc.Bacc("TRN2", target_bir_lowering=False, debug=False)
    x = nc.dram_tensor("x", [BPC, H, W, C], F32, kind="ExternalInput").ap()
    y = nc.dram_tensor("y", [BPC, H, W, C], F32, kind="ExternalOutput").ap()
    with tile.TileContext(nc) as tc:
        _build_kernel(tc, y, x)
    nc.compile()
    _CACHE["nc"] = nc
    return nc


def run(input_batch, **spmd_kwargs):
    nc = _build()
    in_maps = [
        {"x": np.ascontiguousarray(input_batch[i * BPC:(i + 1) * BPC])}
        for i in range(NCORES)
    ]
    res = run_bass_kernel_spmd(nc, in_maps, list(range(NCORES)), **spmd_kwargs)
    out = np.concatenate([r["y"] for r in res.results], axis=0)
    return out, res


def kernel(input_batch):
    out, _ = run(np.asarray(input_batch))
    return out


# revision 14
# speedup vs baseline: 1.4933x; 1.4933x over previous
"""3x3 median filter (reflect padding) on Trainium2, 8-core data parallel.

Layout (per core, 4 images):
  partition p = b*32 + g
    b in 0..3  : image index within the core's batch shard
    g in 0..31 : group of 7 consecutive output rows
  Work is split into 2 row-chunks per group (3 + 4 output rows).  Each
  partition's slab holds (R+2) full-width rows of (224+2)px x 3ch fp32,
  so vertical (stride F) and horizontal (stride 3) neighbor access are
  free-dim offsets and each DRAM row moves as one contiguous 2688B run.

Median of 9 = med3( max3(col_lows), med3(col_meds), min3(col_highs) )
with each vertical column triple sorted once and shared across the three
horizontally adjacent windows.

Loads/stores are spread over three DMA streams (SP + ACT hardware DGE
queues and the GPSIMD software DGE).
"""

import sys

if "/opt/trn_rl_repo" not in sys.path:
    sys.path.insert(0, "/opt/trn_rl_repo")

import numpy as np

import concourse.bass as bass  # noqa: F401
import concourse.tile as tile
from concourse import bacc, mybir
from concourse.ap import AP
from concourse.bass_utils import run_bass_kernel_spmd

F32 = mybir.dt.float32
MIN = mybir.AluOpType.min
MAX = mybir.AluOpType.max

B, H, W, C = 32, 224, 224, 3
NCORES = 8
BPC = B // NCORES      # 4 images per core
NG, GR = 32, 7         # row-groups per image, rows per group
CHUNK_ROWS = (3, 4)    # output rows per chunk within each group
F = (W + 2) * C        # 678 floats per slab row incl. horizontal halo
SC = W * C             # 672 output floats per row
WC = W * C             # 672
IMG = H * WC

_CACHE = {}


def _build_kernel(tc, y, x):
    nc = tc.nc

    with tc.tile_pool(name="sb", bufs=1) as sb:
        r0g = 0  # first output row of this chunk within its group
        for chunk, R in enumerate(CHUNK_ROWS):
            SRR = R + 2
            S = sb.tile([128, SRR, F], F32, tag="s", bufs=2, name=f"S{chunk}")

            # ---- loads -------------------------------------------------
            # Partition p's slab = image rows 7g+r0g-1 .. 7g+r0g+R, where
            # linear(p) = p*GR*WC addresses (b,g) jointly (perfect nest).
            # The bulk covers the rows that are in-bounds for EVERY p in
            # one 128-partition instruction; the one remaining halo row
            # comes from a 127-partition instruction; the single image-
            # boundary partition gets a reflected 1-row DMA.
            PS = GR * WC  # 4704: per-partition linear stride
            first = chunk == 0
            qa, qb = (nc.sync, nc.scalar) if first else (nc.scalar, nc.sync)
            if first:
                # bulk rows 7g .. 7g+R into slab rows 1..R+1
                for h, q in ((0, qa), (1, qb)):
                    src = AP(x.tensor, h * 64 * PS + r0g * WC,
                             [[PS, 64], [WC, SRR - 1], [1, WC]])
                    q.dma_start(S[h * 64:(h + 1) * 64, 1:SRR, 3:675], src)
                # top halo row 7g-1 for p>=1 into slab row 0
                for h, q in ((0, qb), (1, qa)):
                    ps = 1 if h == 0 else 64
                    n = (h + 1) * 64 - ps
                    src = AP(x.tensor, ps * PS + (r0g - 1) * WC,
                             [[PS, n], [1, WC]])
                    q.dma_start(S[ps:ps + n, 0:1, 3:675], src)
                # image-boundary partitions: halo = own image's row 1
                # (p=0 was skipped by the halo DMA; p=32/64/96 got the
                # previous image's row 223 and are overwritten here)
                for b in range(BPC):
                    r = AP(x.tensor, b * IMG + (r0g + 1) * WC, [[1, WC]])
                    qb.dma_start(S[b * 32:b * 32 + 1, 0:1, 3:675], r)
            else:
                # bulk rows 7g+r0g-1 .. 7g+r0g+R-1 into slab rows 0..R
                for h, q in ((0, qa), (1, qb)):
                    src = AP(x.tensor, h * 64 * PS + (r0g - 1) * WC,
                             [[PS, 64], [WC, SRR - 1], [1, WC]])
                    q.dma_start(S[h * 64:(h + 1) * 64, 0:SRR - 1, 3:675], src)
                # bottom halo row 7g+r0g+R for p<=126 into slab row R+1
                for h, q in ((0, qb), (1, qa)):
                    n = 64 if h == 0 else 63
                    src = AP(x.tensor, h * 64 * PS + (r0g + R) * WC,
                             [[PS, n], [1, WC]])
                    q.dma_start(S[h * 64:h * 64 + n, SRR - 1:SRR, 3:675], src)
                # image-boundary partitions: halo = own image's row 222
                # (p=127 was skipped by the halo DMA; p=31/63/95 got the
                # next image's row 0 and are overwritten here)
                for b in range(BPC):
                    r = AP(x.tensor, b * IMG + (H - 2) * WC, [[1, WC]])
                    qa.dma_start(
                        S[b * 32 + 31:b * 32 + 32, SRR - 1:SRR, 3:675], r)

            # horizontal reflect: col -1 <- col 1, col 224 <- col 222
            nc.vector.tensor_copy(S[:, :, 0:3], S[:, :, 6:9])
            nc.vector.tensor_copy(S[:, :, 675:678], S[:, :, 669:672])

            # ---- stage 1: vertical column sort -------------------------
            P = sb.tile([128, R, F], F32, tag="p", name=f"P{chunk}")
            Q = sb.tile([128, R, F], F32, tag="q", name=f"Q{chunk}")
            nc.vector.tensor_tensor(P[:], S[:, 0:R, :], S[:, 1:R + 1, :], MIN)
            nc.vector.tensor_tensor(Q[:], S[:, 0:R, :], S[:, 1:R + 1, :], MAX)

            LO = sb.tile([128, R, F], F32, tag="lo", name=f"LO{chunk}")
            T = sb.tile([128, R, F], F32, tag="t", name=f"T{chunk}")
            nc.vector.tensor_tensor(LO[:], P[:], S[:, 2:SRR, :], MIN)
            nc.vector.tensor_tensor(T[:], Q[:], S[:, 2:SRR, :], MIN)
            # MED (in T): max(P, min(Q, S+2))
            nc.vector.tensor_tensor(T[:], P[:], T[:], MAX)
            # HI (in Q): max(Q, S+2)
            nc.vector.tensor_tensor(Q[:], Q[:], S[:, 2:SRR, :], MAX)
            HI = Q

            # ---- stage 2: horizontal merge -----------------------------
            U = sb.tile([128, R, F - 3], F32, tag="u", name=f"U{chunk}")
            nc.vector.tensor_tensor(U[:], LO[:, :, 0:F - 3], LO[:, :, 3:F], MAX)
            nc.vector.tensor_tensor(U[:, :, 0:SC], U[:, :, 0:SC],
                                    LO[:, :, 6:F], MAX)
            A = U  # max3 of lows

            V = sb.tile([128, R, F - 3], F32, tag="v", name=f"V{chunk}")
            nc.vector.tensor_tensor(V[:], HI[:, :, 0:F - 3], HI[:, :, 3:F], MIN)
            nc.vector.tensor_tensor(V[:, :, 0:SC], V[:, :, 0:SC],
                                    HI[:, :, 6:F], MIN)
            Cc = V  # min3 of highs

            Sm = sb.tile([128, R, F - 3], F32, tag="sm", name=f"Sm{chunk}")
            Tm = sb.tile([128, R, F - 3], F32, tag="tm", name=f"Tm{chunk}")
            nc.vector.tensor_tensor(Sm[:], T[:, :, 0:F - 3], T[:, :, 3:F], MIN)
            nc.vector.tensor_tensor(Tm[:], T[:, :, 0:F - 3], T[:, :, 3:F], MAX)
            nc.vector.tensor_tensor(Tm[:, :, 0:SC], Tm[:, :, 0:SC],
                                    T[:, :, 6:F], MIN)
            nc.vector.tensor_tensor(Sm[:, :, 0:SC], Sm[:, :, 0:SC],
                                    Tm[:, :, 0:SC], MAX)
            Bm = Sm  # med3 of meds

            # ---- final med3(A, B, C) -----------------------------------
            M1 = sb.tile([128, R, SC], F32, tag="m1", bufs=2, name=f"M1{chunk}")
            nc.vector.tensor_tensor(M1[:], A[:, :, 0:SC], Bm[:, :, 0:SC], MIN)
            nc.vector.tensor_tensor(A[:, :, 0:SC], A[:, :, 0:SC],
                                    Bm[:, :, 0:SC], MAX)
            nc.vector.tensor_tensor(Cc[:, :, 0:SC], A[:, :, 0:SC],
                                    Cc[:, :, 0:SC], MIN)
            nc.vector.tensor_tensor(M1[:], M1[:], Cc[:, :, 0:SC], MAX)

            # ---- store -------------------------------------------------
            for h, q in ((0, qb), (1, qa)):
                dst = AP(y.tensor, h * 64 * PS + r0g * WC,
                         [[PS, 64], [WC, R], [1, SC]])
                q.dma_start(dst, M1[h * 64:(h + 1) * 64, :, :])

            r0g += R


def _build():
    if "nc" in _CACHE:
        return _CACHE["nc"]
    nc = bacc.Bacc("TRN2", target_bir_lowering=False, debug=False)
    x = nc.dram_tensor("x", [BPC, H, W, C], F32, kind="ExternalInput").ap()
    y = nc.dram_tensor("y", [BPC, H, W, C], F32, kind="ExternalOutput").ap()
    with tile.TileContext(nc) as tc:
        _build_kernel(tc, y, x)
    nc.compile()
    _CACHE["nc"] = nc
    return nc


def run(input_batch, **spmd_kwargs):
    nc = _build()
    in_maps = [
        {"x": np.ascontiguousarray(input_batch[i * BPC:(i + 1) * BPC])}
        for i in range(NCORES)
    ]
    res = run_bass_kernel_spmd(nc, in_maps, list(range(NCORES)), **spmd_kwargs)
    out = np.concatenate([r["y"] for r in res.results], axis=0)
    return out, res


def kernel(input_batch):
    out, _ = run(np.asarray(input_batch))
    return out
